# revision 11
# baseline (speedup 1.0000x reference)
"""Trainium2 Bass kernel for nn_CANN_75857712382071 — v2.

Single-head self-attention (B=32, A=2048, D=128) with scalar output
projection, algebraically collapsed (see baseline notes):

    out[b,q] = (sum_k E * w') / (sum_k E)
    E[k,q]   = exp(scale * S[k,q] + SHIFT),  S = zT_k-block^T @ UT
    UT       = M^T zT + gw    (M = Wq^T Wk, gw = Wk^T bq)
    w'[k]    = z_k . (Wv^T Wo^T) + (bv.Wo + bo)

v3 changes vs v2 (which precomputed zT/UT/wl on host and split exp
between ScalarE ACT and VectorE Schraudolph):
  * nd reduction matmuls are COL-TILED: the 4 query chunks of one key
    tile run CONCURRENTLY in 4 distinct PE column groups
    (tile_position=(0, 32c)), accumulating into one PSUM bank
    [128, 512] at partition bases 0/32/64/96.  This cuts nd PE cost
    from 4x512 streaming cycles to ~1x512 per key tile (PE busy was
    106us, ~52us of it nd).
  * nd PSUM double-buffered across batches (2 banks); scores keep
    3x [128,1024] (6 banks) = 8 banks total.
  * exp halves are weight-round-robined 17:15 (ACT:DVE) since
    measured ACT 1024-col = 1113ns < DVE = 1223ns.
  * single finale copy per batch ([98,512]) + 2 strided DMAs.
  * longer PE warmup (HAM stayed at K=4/8 for 15us in the v2 trace).

Data-parallel over batch: 4 batches per core on 8 NeuronCores.
"""

import sys
import types

import numpy as np

N_CORES = 8
B, A, D = 32, 2048, 128
B_PER = B // N_CORES
SCALE = float(D) ** -0.5
SHIFT = -1.0                      # uniform logit shift (cancels in ratio)
NT = A // 128                     # 16 key tiles per batch
SCH_C = 7.42                      # Schraudolph bias tuning constant
SCH_A = 128.0 / float(np.log(2.0))
DVE_HALVES = 15                   # of 32 exp half-tiles per batch on DVE


def _dve_half(tk, h):
    """Strict ACT/DVE alternation (15 of 32 halves on DVE).

    Alternation avoids same-engine double-runs that stall the 3-buffer
    PSUM ring; the one extra ACT half sits at the batch end, where DVE
    is busy with the finale copy anyway.
    """
    i = 2 * tk + h
    return (i % 2 == 1) and i != 31


def _install_axon_shim():
    """Allow run_bass_kernel_spmd(trace=True) to NTFF-profile under axon."""
    try:
        import antenv  # noqa: F401
    except ImportError:
        return
    if "antenv.axon_hooks" not in sys.modules:
        mod = types.ModuleType("antenv.axon_hooks")
        _hook = [None]
        mod.set_axon_ntff_profile_hook = lambda h: _hook.__setitem__(0, h)
        mod.get_axon_ntff_profile_hook = lambda: _hook[0]
        sys.modules["antenv.axon_hooks"] = mod
    from antenv.axon_hooks import (
        get_axon_ntff_profile_hook,
        set_axon_ntff_profile_hook,
    )
    if get_axon_ntff_profile_hook() is None:
        try:
            from trn_agent_boot.trn_boot import _ntff_profile_via_ctypes
            set_axon_ntff_profile_hook(
                _ntff_profile_via_ctypes("/opt/axon/libaxon_pjrt.so"))
        except Exception:
            pass
    try:
        from concourse import bass_utils
        bass_utils.upload_artifacts = lambda tmpdir: tmpdir
    except Exception:
        pass


def _build_program():
    import concourse.bacc as bacc
    import concourse.mybir as mybir
    import concourse.tile as tile

    f32 = mybir.dt.float32
    bf16 = mybir.dt.bfloat16
    i16 = mybir.dt.int16
    AF = mybir.ActivationFunctionType
    MULT = mybir.AluOpType.mult
    ADD = mybir.AluOpType.add

    sch_s1 = SCH_A * SCALE
    sch_s2 = 127.0 * 128.0 - SCH_C + SCH_A * SHIFT

    nc = bacc.Bacc("TRN2", target_bir_lowering=False, debug=False,
                   num_devices=N_CORES, num_swdge_queues=2)

    zT_d = nc.dram_tensor("zT", [B_PER, D, A], bf16, kind="ExternalInput").ap()
    ut_d = nc.dram_tensor("ut", [B_PER, D, A], bf16, kind="ExternalInput").ap()
    wl_d = nc.dram_tensor("wl", [B_PER, 128, 2 * NT], bf16,
                          kind="ExternalInput").ap()
    # numerator/denominator pairs; the final division happens on the host
    out_d = nc.dram_tensor("ond", [B_PER, 2, A], f32,
                           kind="ExternalOutput").ap()

    NC_ = 4                        # nd accumulator chunks (512 queries each)

    with tile.TileContext(nc) as tc:
        with (
            tc.tile_pool(name="sb", bufs=1) as sb,
            tc.tile_pool(name="ps", bufs=3, space="PSUM") as ps,
        ):
            # ACT table warmup + PE HAM pre-warm on junk data while the
            # first DMAs are in flight.
            shift_col = sb.tile([D, 1], f32)
            nc.vector.memset(shift_col[:], SHIFT)
            warm = sb.tile([D, 1], f32)
            nc.vector.memset(warm[:], 0.0)
            nc.scalar.activation(warm[:], warm[:], AF.Exp, scale=0.0,
                                 bias=shift_col[:])
            junk = sb.tile([128, 512], bf16)
            nc.vector.memset(junk[:, 0:8], 0.0)
            pjunk = ps.tile([128, 512], f32, name="pjunk", tag="sc")
            for i in range(10):
                nc.tensor.matmul(pjunk[:], junk[:, 0:128], junk[:],
                                 start=True, stop=True)

            st = {}

            def emit_in_dmas(b):
                s = st.setdefault(b, {})
                zt = sb.tile([D, A], bf16, name=f"zt{b}", tag="zt", bufs=2)
                ut = sb.tile([D, A], bf16, name=f"ut{b}", tag="ut", bufs=2)
                wl = sb.tile([128, 2 * NT], bf16, name=f"wl{b}", tag="wl",
                             bufs=2)
                # split across queues/engines for bandwidth
                for q in range(4):
                    sl = slice(q * 512, (q + 1) * 512)
                    nc.sync.dma_start(zt[:, sl], zT_d[b][:, sl])
                    nc.gpsimd.dma_start(ut[:, sl], ut_d[b][:, sl])
                nc.sync.dma_start(wl[:], wl_d[b])
                s["zt"], s["ut"], s["wl"] = zt, ut, wl

            def emit_scores_tk(b, tk):
                """scores for key tile tk -> exp -> eT bf16 [128, A]."""
                s = st[b]
                lhs = s["zt"][:, tk * 128:(tk + 1) * 128]
                eT = sb.tile([128, A], bf16, name=f"e{b}_{tk}", tag="eT",
                             bufs=10)
                for h in range(2):
                    ps_t = ps.tile([128, 1024], f32, name=f"s{b}_{tk}_{h}",
                                   tag="sc")
                    for j in range(2):
                        o = h * 1024 + j * 512
                        nc.tensor.matmul(ps_t[:, j * 512:(j + 1) * 512], lhs,
                                         s["ut"][:, o:o + 512],
                                         start=True, stop=True)
                    dst = eT[:, h * 1024:(h + 1) * 1024]
                    # weighted round-robin half assignment: each query column
                    # gets a near-even mix of ACT-exp and DVE Schraudolph
                    # keys; ACT takes 17/32 since it is faster per column.
                    if _dve_half(tk, h):
                        nc.vector.tensor_scalar(
                            dst.bitcast(i16), ps_t[:], sch_s1, sch_s2,
                            MULT, ADD)
                    else:
                        nc.scalar.activation(dst, ps_t[:], AF.Exp,
                                             bias=shift_col[:], scale=SCALE)
                return eT

            def emit_nd_tk(b, tk, eT):
                # nd chunk c accumulates at partition base 32*c of ONE psum
                # bank; the 4 matmuls are col-tiled (tile_position) so they
                # run CONCURRENTLY in 4 distinct PE column groups.
                s = st[b]
                wlt = s["wl"][:, 2 * tk:2 * tk + 2]
                for c in range(NC_):
                    nc.tensor.matmul(
                        s["nd"][32 * c:32 * c + 2, :], wlt,
                        eT[:, c * 512:(c + 1) * 512],
                        start=(tk == 0), stop=(tk == NT - 1),
                        tile_position=(0, 32 * c))

            def emit_finale(b):
                s = st[b]
                # copy the nd psum bank (incl. junk partitions between the
                # per-chunk rows) to sbuf, then 2 strided DMAs assemble the
                # num/den rows in DRAM.
                ndall = sb.tile([98, 512], f32, name=f"ndall{b}",
                                tag="ndall", bufs=2)
                nc.vector.tensor_copy(ndall[:], s["nd"][0:98, :])
                for r in range(2):         # 0=num row, 1=den row
                    nc.sync.dma_start(
                        out_d[b, r:r + 1, :]
                        .rearrange("one (c q) -> (one c) q", c=4),
                        ndall[r:98:32, :])
                st.pop(b)

            # ---- software pipeline over batches ----
            emit_in_dmas(0)
            pend = []   # [(b, tk, eT)] awaiting nd emission

            def flush_pend(keep):
                while len(pend) > keep:
                    pb, ptk, peT = pend.pop(0)
                    emit_nd_tk(pb, ptk, peT)
                    if ptk == NT - 1:
                        emit_finale(pb)

            for b in range(B_PER):
                s = st[b]
                s["nd"] = ps.tile([128, 512], f32, name=f"nd{b}",
                                  tag="nd", bufs=2)
                nxt = b + 1 if b + 1 < B_PER else None
                last = nxt is None
                for tk in range(NT):
                    eT = emit_scores_tk(b, tk)
                    pend.append((b, tk, eT))
                    flush_pend(1 if (last and tk >= NT - 3) else 3)
                    if nxt is not None and tk == 8:
                        emit_in_dmas(nxt)
            flush_pend(0)

    nc.compile()
    return nc


def run(inputs: dict, trace: bool = False):
    _install_axon_shim()
    import ml_dtypes
    from concourse.bass_utils import run_bass_kernel_spmd

    z = np.asarray(inputs["z"], dtype=np.float32)
    Wq = np.asarray(inputs["Wq"], dtype=np.float64)
    bq = np.asarray(inputs["bq"], dtype=np.float64)
    Wk = np.asarray(inputs["Wk"], dtype=np.float64)
    Wv = np.asarray(inputs["Wv"], dtype=np.float64)
    bv = np.asarray(inputs["bv"], dtype=np.float64)
    Wo = np.asarray(inputs["Wo"], dtype=np.float64)
    bo = np.asarray(inputs["bo"], dtype=np.float64)

    # host-side algebra (exact in float64, then f32)
    M = (Wq.T @ Wk).astype(np.float64)
    gw = (Wk.T @ bq).astype(np.float64)
    wvec = (Wv.T @ Wo[0]).astype(np.float64)
    cbo = float(bv @ Wo[0] + bo[0])

    z64 = z.astype(np.float64)
    # zT [B, D, A], UT = M^T zT + gw, w' = z.wvec + cbo
    zT = np.ascontiguousarray(z64.transpose(0, 2, 1))
    # UT[d, a] = sum_e M[e, d] * zT[e, a]  (= M^T zT, matching the
    # baseline's lhsT=M stationary matmul) + gw broadcast
    UT = np.einsum("ed,bea->bda", M, zT, optimize=True) + gw[None, :, None]
    wprime = z64 @ wvec + cbo                      # [B, A]
    wl = np.zeros((B, 128, 2 * NT), dtype=np.float64)
    wl[:, :, 1::2] = 1.0
    # wl[b, p, 2t] = w'[b, t*128+p]
    wl[:, :, 0::2] = wprime.reshape(B, NT, 128).transpose(0, 2, 1)

    zT_bf = zT.astype(ml_dtypes.bfloat16)
    UT_bf = UT.astype(ml_dtypes.bfloat16)
    wl_bf = wl.astype(ml_dtypes.bfloat16)

    nc = _build_program()

    in_maps = []
    for c in range(N_CORES):
        sl = slice(c * B_PER, (c + 1) * B_PER)
        in_maps.append({
            "zT": zT_bf[sl],
            "ut": UT_bf[sl],
            "wl": wl_bf[sl],
        })
    res = run_bass_kernel_spmd(nc, in_maps, core_ids=list(range(N_CORES)),
                               trace=trace)
    nd = np.concatenate([res.results[c]["ond"] for c in range(N_CORES)],
                        axis=0)                     # [B, 2, A]
    out = nd[:, 0, :] / nd[:, 1, :]
    return out.reshape(B, A, 1).astype(np.float32), res


def kernel(**inputs) -> np.ndarray:
    out, _ = run(inputs, trace=False)
    return out



# revision 14
# speedup vs baseline: 1.0311x; 1.0311x over previous
"""Trainium2 Bass kernel for nn_CANN_75857712382071 — v2.

Single-head self-attention (B=32, A=2048, D=128) with scalar output
projection, algebraically collapsed (see baseline notes):

    out[b,q] = (sum_k E * w') / (sum_k E)
    E[k,q]   = exp(scale * S[k,q] + SHIFT),  S = zT_k-block^T @ UT
    UT       = M^T zT + gw    (M = Wq^T Wk, gw = Wk^T bq)
    w'[k]    = z_k . (Wv^T Wo^T) + (bv.Wo + bo)

v3 changes vs v2 (which precomputed zT/UT/wl on host and split exp
between ScalarE ACT and VectorE Schraudolph):
  * nd reduction matmuls are COL-TILED: the 4 query chunks of one key
    tile run CONCURRENTLY in 4 distinct PE column groups
    (tile_position=(0, 32c)), accumulating into one PSUM bank
    [128, 512] at partition bases 0/32/64/96.  This cuts nd PE cost
    from 4x512 streaming cycles to ~1x512 per key tile (PE busy was
    106us, ~52us of it nd).
  * nd PSUM double-buffered across batches (2 banks); scores keep
    3x [128,1024] (6 banks) = 8 banks total.
  * exp halves are weight-round-robined 17:15 (ACT:DVE) since
    measured ACT 1024-col = 1113ns < DVE = 1223ns.
  * single finale copy per batch ([98,512]) + 2 strided DMAs.
  * longer PE warmup (HAM stayed at K=4/8 for 15us in the v2 trace).

Data-parallel over batch: 4 batches per core on 8 NeuronCores.
"""

import sys
import types

import numpy as np

N_CORES = 8
B, A, D = 32, 2048, 128
B_PER = B // N_CORES
SCALE = float(D) ** -0.5
SHIFT = -1.0                      # uniform logit shift (cancels in ratio)
NT = A // 128                     # 16 key tiles per batch
SCH_C = 7.42                      # Schraudolph bias tuning constant
SCH_A = 128.0 / float(np.log(2.0))
DVE_HALVES = 15                   # of 32 exp half-tiles per batch on DVE


def _dve_half(tk, h):
    """Strict ACT/DVE alternation (15 of 32 halves on DVE).

    Alternation avoids same-engine double-runs that stall the 3-buffer
    PSUM ring; the one extra ACT half sits at the batch end, where DVE
    is busy with the finale copy anyway.
    """
    i = 2 * tk + h
    return (i % 2 == 1) and i != 31


def _install_axon_shim():
    """Allow run_bass_kernel_spmd(trace=True) to NTFF-profile under axon."""
    try:
        import antenv  # noqa: F401
    except ImportError:
        return
    if "antenv.axon_hooks" not in sys.modules:
        mod = types.ModuleType("antenv.axon_hooks")
        _hook = [None]
        mod.set_axon_ntff_profile_hook = lambda h: _hook.__setitem__(0, h)
        mod.get_axon_ntff_profile_hook = lambda: _hook[0]
        sys.modules["antenv.axon_hooks"] = mod
    from antenv.axon_hooks import (
        get_axon_ntff_profile_hook,
        set_axon_ntff_profile_hook,
    )
    if get_axon_ntff_profile_hook() is None:
        try:
            from trn_agent_boot.trn_boot import _ntff_profile_via_ctypes
            set_axon_ntff_profile_hook(
                _ntff_profile_via_ctypes("/opt/axon/libaxon_pjrt.so"))
        except Exception:
            pass
    try:
        from concourse import bass_utils
        bass_utils.upload_artifacts = lambda tmpdir: tmpdir
    except Exception:
        pass


def _build_program():
    import concourse.bacc as bacc
    import concourse.mybir as mybir
    import concourse.tile as tile

    f32 = mybir.dt.float32
    bf16 = mybir.dt.bfloat16
    i16 = mybir.dt.int16
    AF = mybir.ActivationFunctionType
    MULT = mybir.AluOpType.mult
    ADD = mybir.AluOpType.add

    sch_s1 = SCH_A * SCALE
    sch_s2 = 127.0 * 128.0 - SCH_C + SCH_A * SHIFT

    nc = bacc.Bacc("TRN2", target_bir_lowering=False, debug=False,
                   num_devices=N_CORES, num_swdge_queues=4)

    zT_d = nc.dram_tensor("zT", [B_PER, D, A], bf16, kind="ExternalInput").ap()
    ut_d = nc.dram_tensor("ut", [B_PER, D, A], bf16, kind="ExternalInput").ap()
    wl_d = nc.dram_tensor("wl", [B_PER, 128, 2 * NT], bf16,
                          kind="ExternalInput").ap()
    # numerator/denominator pairs; the final division happens on the host
    out_d = nc.dram_tensor("ond", [B_PER, 2, A], f32,
                           kind="ExternalOutput").ap()

    NC_ = 4                        # nd accumulator chunks (512 queries each)

    with tile.TileContext(nc) as tc:
        with (
            tc.tile_pool(name="sb", bufs=1) as sb,
            tc.tile_pool(name="ps", bufs=3, space="PSUM") as ps,
        ):
            # ACT table warmup + PE HAM pre-warm on junk data while the
            # first DMAs are in flight.
            shift_col = sb.tile([D, 1], f32)
            nc.vector.memset(shift_col[:], SHIFT)
            warm = sb.tile([D, 1], f32)
            nc.vector.memset(warm[:], 0.0)
            nc.scalar.activation(warm[:], warm[:], AF.Exp, scale=0.0,
                                 bias=shift_col[:])
            junk = sb.tile([128, 512], bf16)
            nc.vector.memset(junk[:, 0:8], 0.0)
            pjunk = ps.tile([128, 512], f32, name="pjunk", tag="sc")
            for i in range(10):
                nc.tensor.matmul(pjunk[:], junk[:, 0:128], junk[:],
                                 start=True, stop=True)

            st = {}

            def alloc_in_tiles(b):
                s = st.setdefault(b, {})
                s["zt"] = sb.tile([D, A], bf16, name=f"zt{b}", tag="zt",
                                  bufs=2)
                s["ut"] = sb.tile([D, A], bf16, name=f"ut{b}", tag="ut",
                                  bufs=2)
                s["wl"] = sb.tile([128, 2 * NT], bf16, name=f"wl{b}",
                                  tag="wl", bufs=2)

            def emit_in_dma_step(b, step):
                # one chunk per step, spread over the previous batch's key
                # tiles so the DMA queues never pile up mid-batch.
                # steps 0-3: zt chunk (sync/HWDGE) + ut chunk (gpsimd/SWDGE)
                # step 4: wl (sync)
                s = st[b]
                if step < 4:
                    sl = slice(step * 512, (step + 1) * 512)
                    nc.sync.dma_start(s["zt"][:, sl], zT_d[b][:, sl])
                    nc.gpsimd.dma_start(s["ut"][:, sl], ut_d[b][:, sl])
                else:
                    nc.sync.dma_start(s["wl"][:], wl_d[b])

            def emit_in_dmas(b):
                alloc_in_tiles(b)
                for step in range(5):
                    emit_in_dma_step(b, step)

            def emit_scores_tk(b, tk):
                """scores for key tile tk -> exp -> eT bf16 [128, A]."""
                s = st[b]
                lhs = s["zt"][:, tk * 128:(tk + 1) * 128]
                eT = sb.tile([128, A], bf16, name=f"e{b}_{tk}", tag="eT",
                             bufs=10)
                for h in range(2):
                    ps_t = ps.tile([128, 1024], f32, name=f"s{b}_{tk}_{h}",
                                   tag="sc")
                    for j in range(2):
                        o = h * 1024 + j * 512
                        nc.tensor.matmul(ps_t[:, j * 512:(j + 1) * 512], lhs,
                                         s["ut"][:, o:o + 512],
                                         start=True, stop=True)
                    dst = eT[:, h * 1024:(h + 1) * 1024]
                    # weighted round-robin half assignment: each query column
                    # gets a near-even mix of ACT-exp and DVE Schraudolph
                    # keys; ACT takes 17/32 since it is faster per column.
                    if _dve_half(tk, h):
                        nc.vector.tensor_scalar(
                            dst.bitcast(i16), ps_t[:], sch_s1, sch_s2,
                            MULT, ADD)
                    else:
                        nc.scalar.activation(dst, ps_t[:], AF.Exp,
                                             bias=shift_col[:], scale=SCALE)
                return eT

            def emit_nd_tk(b, tk, eT):
                # nd chunk c accumulates at partition base 32*c of ONE psum
                # bank; the 4 matmuls are col-tiled (tile_position) so they
                # run CONCURRENTLY in 4 distinct PE column groups.
                s = st[b]
                wlt = s["wl"][:, 2 * tk:2 * tk + 2]
                for c in range(NC_):
                    nc.tensor.matmul(
                        s["nd"][32 * c:32 * c + 2, :], wlt,
                        eT[:, c * 512:(c + 1) * 512],
                        start=(tk == 0), stop=(tk == NT - 1),
                        tile_position=(0, 32 * c))

            def emit_finale(b):
                s = st[b]
                # copy the nd psum bank (incl. junk partitions between the
                # per-chunk rows) to sbuf, then 2 strided DMAs assemble the
                # num/den rows in DRAM.
                ndall = sb.tile([98, 512], f32, name=f"ndall{b}",
                                tag="ndall", bufs=2)
                nc.vector.tensor_copy(ndall[:], s["nd"][0:98, :])
                for r in range(2):         # 0=num row, 1=den row
                    nc.sync.dma_start(
                        out_d[b, r:r + 1, :]
                        .rearrange("one (c q) -> (one c) q", c=4),
                        ndall[r:98:32, :])
                st.pop(b)

            # ---- software pipeline over batches ----
            emit_in_dmas(0)
            pend = []   # [(b, tk, eT)] awaiting nd emission

            def flush_pend(keep):
                while len(pend) > keep:
                    pb, ptk, peT = pend.pop(0)
                    emit_nd_tk(pb, ptk, peT)
                    if ptk == NT - 1:
                        emit_finale(pb)

            for b in range(B_PER):
                s = st[b]
                s["nd"] = ps.tile([128, 512], f32, name=f"nd{b}",
                                  tag="nd", bufs=2)
                nxt = b + 1 if b + 1 < B_PER else None
                last = nxt is None
                for tk in range(NT):
                    eT = emit_scores_tk(b, tk)
                    pend.append((b, tk, eT))
                    flush_pend(1 if (last and tk >= NT - 3) else 3)
                    if nxt is not None and 4 <= tk <= 8:
                        if tk == 4:
                            alloc_in_tiles(nxt)
                        emit_in_dma_step(nxt, tk - 4)
            flush_pend(0)

    nc.compile()
    return nc


def run(inputs: dict, trace: bool = False):
    _install_axon_shim()
    import ml_dtypes
    from concourse.bass_utils import run_bass_kernel_spmd

    z = np.asarray(inputs["z"], dtype=np.float32)
    Wq = np.asarray(inputs["Wq"], dtype=np.float64)
    bq = np.asarray(inputs["bq"], dtype=np.float64)
    Wk = np.asarray(inputs["Wk"], dtype=np.float64)
    Wv = np.asarray(inputs["Wv"], dtype=np.float64)
    bv = np.asarray(inputs["bv"], dtype=np.float64)
    Wo = np.asarray(inputs["Wo"], dtype=np.float64)
    bo = np.asarray(inputs["bo"], dtype=np.float64)

    # host-side algebra (exact in float64, then f32)
    M = (Wq.T @ Wk).astype(np.float64)
    gw = (Wk.T @ bq).astype(np.float64)
    wvec = (Wv.T @ Wo[0]).astype(np.float64)
    cbo = float(bv @ Wo[0] + bo[0])

    z64 = z.astype(np.float64)
    # zT [B, D, A], UT = M^T zT + gw, w' = z.wvec + cbo
    zT = np.ascontiguousarray(z64.transpose(0, 2, 1))
    # UT[d, a] = sum_e M[e, d] * zT[e, a]  (= M^T zT, matching the
    # baseline's lhsT=M stationary matmul) + gw broadcast
    UT = np.einsum("ed,bea->bda", M, zT, optimize=True) + gw[None, :, None]
    wprime = z64 @ wvec + cbo                      # [B, A]
    wl = np.zeros((B, 128, 2 * NT), dtype=np.float64)
    wl[:, :, 1::2] = 1.0
    # wl[b, p, 2t] = w'[b, t*128+p]
    wl[:, :, 0::2] = wprime.reshape(B, NT, 128).transpose(0, 2, 1)

    zT_bf = zT.astype(ml_dtypes.bfloat16)
    UT_bf = UT.astype(ml_dtypes.bfloat16)
    wl_bf = wl.astype(ml_dtypes.bfloat16)

    nc = _build_program()

    in_maps = []
    for c in range(N_CORES):
        sl = slice(c * B_PER, (c + 1) * B_PER)
        in_maps.append({
            "zT": zT_bf[sl],
            "ut": UT_bf[sl],
            "wl": wl_bf[sl],
        })
    res = run_bass_kernel_spmd(nc, in_maps, core_ids=list(range(N_CORES)),
                               trace=trace)
    nd = np.concatenate([res.results[c]["ond"] for c in range(N_CORES)],
                        axis=0)                     # [B, 2, A]
    out = nd[:, 0, :] / nd[:, 1, :]
    return out.reshape(B, A, 1).astype(np.float32), res


def kernel(**inputs) -> np.ndarray:
    out, _ = run(inputs, trace=False)
    return out

